# revision 1
# baseline (speedup 1.0000x reference)
"""Bahdanau attention kernel for Trainium2 (8 NeuronCores, SPMD data-parallel).

Reference computation (per batch b):
    f_proj = features[b] @ W1_w + W1_b            # [T, U]
    h_proj = hidden[b] @ W2_w + W2_b              # [U]
    score  = tanh(f_proj + h_proj) @ V_w + V_b    # [T]
    attn   = softmax(score)                       # [T]
    context[b] = sum_t attn[t] * features[b, t]   # [D]

Sharding: data-parallel over batch (64 batches / 8 cores = 8 per core),
weights replicated.

Per-core dataflow (bf16 matmul operands, fp32 accumulation everywhere):
  - F loads HBM->SBUF with an inline fp32->bf16 cast (gpsimd casting DMA,
    one descriptor per 512-t chunk), then XBAR transpose DMAs
    (dma_start_transpose, 16x128 tiles) produce F^T [128(d), dc, t]
    directly in SBUF.  The PE never touches F for transposition and
    there are no PSUM->SBUF repack copies.
  - main matmul computes f_proj TRANSPOSED: [u(part), t(free)] =
    W1_chunk^T @ F^T, so the (W1_b + h_proj) bias is a per-partition
    scalar that fuses into the ACT Tanh instruction (bf16 out).
  - score uses a REPLICATED stationary V_rep[u, m] = V[u], so the PE
    produces score broadcast across all 128 partitions in one shot.
    ACT Exp turns that into e_bc [128, t] bf16 in SBUF; DVE row-reduces
    partition 0 for sum(e).  No max-subtraction: scores are O(3) here
    and bf16/fp32 exp has huge range.
  - context needs no PE work either: DVE's fused tensor_tensor_reduce
    computes ctx[d] += sum_t F^T[d, t] * e_bc[d, t] over the resident
    F^T tiles (all-bf16 operands -> 2x DVE mode).  Final per-batch
    scale by 1/sum(e), one tiny all-fp32 PE transpose to [dc, 128], and
    a 4-descriptor DMA out.

The h_proj/bias setup path stays entirely fp32 (tiny work, no precision
loss): W2/hidden are loaded as fp32 and processed with fp32 matmuls.
"""

import sys

for _p in ("/opt/trn_rl_repo", "/opt/pypackages"):
    if _p not in sys.path:
        sys.path.insert(0, _p)

import numpy as np

B, T, D, U = 64, 2048, 512, 512
NCORES = 8
BPC = B // NCORES          # batches per core
PART = 128
DC = D // PART             # 4 contraction chunks
UC = U // PART             # 4 u chunks
TCHUNK = 512               # t columns processed per main-matmul group
TILES_PER_CHUNK = TCHUNK // PART          # 4
NCHUNKS = (BPC * T) // TCHUNK             # 32
CHUNKS_PER_BATCH = T // TCHUNK            # 4

MM_DT_NAME = "bfloat16"    # dtype tag for matmul operands


_BUILD_CACHE = {}


def build_bass(mm_dt_name=MM_DT_NAME):
    """Build + compile the per-core Bass program (same on all cores)."""
    if mm_dt_name in _BUILD_CACHE:
        return _BUILD_CACHE[mm_dt_name]

    import concourse.mybir as mybir
    import concourse.tile as tile
    from concourse import bacc
    from concourse.bass import ts
    from concourse.masks import make_identity

    f32 = mybir.dt.float32
    mdt = getattr(mybir.dt, mm_dt_name)
    ACT = mybir.ActivationFunctionType
    AX = mybir.AxisListType
    ALU = mybir.AluOpType

    nc = bacc.Bacc("TRN2", target_bir_lowering=False, debug=False)

    feat = nc.dram_tensor("features", [BPC, T, D], f32, kind="ExternalInput")
    hid = nc.dram_tensor("hidden", [BPC, D], f32, kind="ExternalInput")
    w1 = nc.dram_tensor("W1_w", [D, U], f32, kind="ExternalInput")
    b1 = nc.dram_tensor("W1_b", [U], f32, kind="ExternalInput")
    w2 = nc.dram_tensor("W2_w", [D, U], f32, kind="ExternalInput")
    b2 = nc.dram_tensor("W2_b", [U], f32, kind="ExternalInput")
    vw = nc.dram_tensor("V_w", [U, 1], f32, kind="ExternalInput")
    vb = nc.dram_tensor("V_b", [1], f32, kind="ExternalInput")
    out = nc.dram_tensor("context", [BPC, D], f32, kind="ExternalOutput")

    with tile.TileContext(nc) as tc:
        with (
            tc.tile_pool(name="consts", bufs=1) as consts,
            tc.tile_pool(name="fpool", bufs=4) as fpool,
            tc.tile_pool(name="fpre32", bufs=4) as fpre32,
            tc.tile_pool(name="ftb", bufs=4) as ftb,
            tc.tile_pool(name="tanh", bufs=3) as tanhp,
            tc.tile_pool(name="small", bufs=3) as small,
            tc.tile_pool(name="ebc", bufs=2) as ebcp,
            tc.tile_pool(name="pscratch", bufs=2) as pscratch,
            tc.tile_pool(name="ctxp", bufs=2) as ctxp,
            tc.tile_pool(name="outp", bufs=2) as outp,
            tc.tile_pool(name="ps_mm", bufs=3, space="PSUM") as ps_mm,
            tc.tile_pool(name="ps_t", bufs=3, space="PSUM") as ps_t,
            tc.tile_pool(name="ps_s", bufs=2, space="PSUM") as ps_s,
        ):
            # ---------------- constants / setup ----------------
            ident_f32 = consts.tile([PART, PART], f32)
            make_identity(nc, ident_f32)
            ident = consts.tile([PART, PART], mdt)
            nc.vector.tensor_copy(ident, ident_f32)
            ones128 = consts.tile([PART, PART], f32)
            nc.vector.memset(ones128, 1.0)

            # small consts first so the h_proj setup can start within ~1us
            hid_sb = consts.tile([BPC, D], f32)
            nc.sync.dma_start(out=hid_sb, in_=hid.ap())
            v_f32 = consts.tile([PART, UC], f32)
            nc.sync.dma_start(out=v_f32, in_=vw.ap().rearrange("(c p) one -> p (c one)", p=PART))
            vb_sb = consts.tile([1, 1], f32)
            nc.sync.dma_start(out=vb_sb, in_=vb.ap().rearrange("(one x) -> one x", one=1))
            b1_sb = consts.tile([PART, UC], f32)
            nc.sync.dma_start(out=b1_sb, in_=b1.ap().rearrange("(c p) -> p c", p=PART))
            b2_sb = consts.tile([PART, UC], f32)
            nc.sync.dma_start(out=b2_sb, in_=b2.ap().rearrange("(c p) -> p c", p=PART))
            b12_sb = consts.tile([PART, UC], f32)
            nc.vector.tensor_add(b12_sb, b1_sb, b2_sb)
            # fp32 side path: W2 for h_proj (scalar queue, overlaps preloads)
            w2_sb = consts.tile([PART, DC, U], f32)
            nc.scalar.dma_start(
                out=w2_sb, in_=w2.ap().rearrange("(c p) u -> p c u", p=PART)
            )

            # preload the first four chunks' F as plain fp32 over the fast
            # HWDGE queues: the gpsimd casting-DMA (SWDGE) pipeline has
            # ~15-18us startup latency, so the first SWDGE chunk (4) gets a
            # long runway.  Per-tile DMAs let the first transposes start as
            # soon as tile j=0 lands.
            NPRE = 4
            preloaded = {}
            pre_dmas = []
            for pch in range(NPRE):
                pb = pch // CHUNKS_PER_BATCH
                pt0 = (pch % CHUNKS_PER_BATCH) * TCHUNK
                f_pre = fpre32.tile(
                    [PART, TILES_PER_CHUNK, D], f32, tag="F32", name=f"f_pre_{pch}"
                )
                for j in range(TILES_PER_CHUNK):
                    eng = nc.sync if (pch * TILES_PER_CHUNK + j) % 2 == 0 else nc.scalar
                    pre_dmas.append(
                        (eng, f_pre[:, j, :],
                         feat.ap()[pb, pt0 + j * PART : pt0 + (j + 1) * PART, :])
                    )
                preloaded[pch] = f_pre
            # chunk 0's slices + W1 go first; later preloads follow behind
            for eng, o, i_ in pre_dmas[:TILES_PER_CHUNK]:
                eng.dma_start(out=o, in_=i_)
            w1_f32 = consts.tile([PART, DC, U], f32)
            nc.sync.dma_start(
                out=w1_f32, in_=w1.ap().rearrange("(c p) u -> p c u", p=PART)
            )
            w1_sb = consts.tile([PART, DC, U], mdt)
            nc.vector.tensor_copy(w1_sb, w1_f32)
            for eng, o, i_ in pre_dmas[TILES_PER_CHUNK:]:
                eng.dma_start(out=o, in_=i_)
            # V replicated across the stationary free dim: the score matmul
            # then emits score broadcast over all 128 output partitions
            v_rep = consts.tile([PART, UC, PART], mdt)
            vb_bc = consts.tile([PART, 1], f32)
            hidT_sb = consts.tile([PART, DC, BPC], f32)
            bias_cols = consts.tile([PART, UC, BPC], f32)

            def emit_setup():
                # all-fp32 h_proj path (tiny); emitted after chunk 0's mains
                # are underway so the PE isn't blocked on these DMAs at start
                for dc in range(DC):
                    ps_h = ps_t.tile([PART, TCHUNK], f32, tag="T", name="ps_h")
                    nc.tensor.transpose(ps_h[:, 0:BPC], hid_sb[:, ts(dc, PART)], ident_f32[0:BPC, 0:BPC])
                    nc.vector.tensor_copy(hidT_sb[:, dc, :], ps_h[:, 0:BPC])

            def emit_setup_b():
                # h_projT[u, b] = sum_dc W2[dc]^T @ hiddenT[dc]  (+W2_b+W1_b)
                for uc in range(UC):
                    ps_h = ps_t.tile([PART, TCHUNK], f32, tag="T", name="ps_h2")
                    for dc in range(DC):
                        nc.tensor.matmul(
                            ps_h[:, 0:BPC],
                            w2_sb[:, dc, ts(uc, PART)],
                            hidT_sb[:, dc, :],
                            start=(dc == 0),
                            stop=(dc == DC - 1),
                        )
                    nc.vector.tensor_scalar_add(
                        bias_cols[:, uc, :], ps_h[:, 0:BPC], b12_sb[:, uc : uc + 1]
                    )
                # V_rep[u, uc, m] = V[uc*128 + u]; V_b broadcast to [128, 1]
                for uc in range(UC):
                    nc.vector.tensor_scalar_mul(
                        v_rep[:, uc, :], ones128, v_f32[:, uc : uc + 1]
                    )
                vb2 = consts.tile([1, 2], f32)
                nc.vector.tensor_copy(vb2[:, 0:1], vb_sb)
                nc.vector.tensor_copy(vb2[:, 1:2], vb_sb)
                ps_vb = ps_t.tile([PART, TCHUNK], f32, tag="T", name="ps_vb")
                nc.tensor.matmul(ps_vb[:, 0:2], ones128[0:1, :], vb2, start=True, stop=True)
                nc.vector.tensor_copy(vb_bc, ps_vb[:, 0:1])

            # ---------------- main loop (epilogue deferred one chunk) ----------------
            prev = None          # chunk state awaiting its score/context stage
            batch_state = {}     # per-batch running-sum / ctx accumulators

            def emit_scores(st):
                b, cib = st["b"], st["cib"]
                if cib == 0:
                    batch_state["s_sb"] = small.tile([PART, CHUNKS_PER_BATCH], f32, tag="ssum", name="s_sb")
                    batch_state["ctx_parts"] = ctxp.tile(
                        [PART, DC * CHUNKS_PER_BATCH], f32, tag="ctxp", name="ctx_parts"
                    )
                s_sb = batch_state["s_sb"]

                # score broadcast [128, t]: every output partition m gets
                # score[t] because the stationary V_rep column m is V itself
                ps_sc = ps_s.tile([PART, TCHUNK], f32, tag="score")
                for uc in range(UC):
                    nc.tensor.matmul(
                        ps_sc,
                        v_rep[:, uc, :],
                        st["tanh"][:, uc, :],
                        start=(uc == 0),
                        stop=(uc == UC - 1),
                    )
                # e = exp(score + V_b) on all 128 partitions -> SBUF bf16;
                # the ACT accumulator gives sum_t(e) per partition for free
                e_bc = ebcp.tile([PART, TCHUNK], mdt, tag="e_bc")
                nc.scalar.activation(
                    e_bc, ps_sc, ACT.Exp, bias=vb_bc, accum_out=s_sb[:, cib : cib + 1]
                )
                st["e_bc"] = e_bc

            def emit_context(st):
                b, cib = st["b"], st["cib"]
                s_sb = batch_state["s_sb"]
                ctx_parts = batch_state["ctx_parts"]
                e_bc = st["e_bc"]
                ftile_big = st["ftile_big"]
                # DVE fused multiply + free-dim reduce (all-bf16 -> 2x mode):
                # ctx_parts[d, dc*4+cib] = sum_t F^T[d, dc, t] * e[t]
                for dc in range(DC):
                    scr = pscratch.tile([PART, TCHUNK], mdt, tag="scr", name="pscr")
                    nc.vector.scalar_tensor_tensor(
                        out=scr,
                        in0=ftile_big[:, dc, :],
                        scalar=1.0,
                        in1=e_bc,
                        op0=ALU.mult,
                        op1=ALU.mult,
                        accum_out=ctx_parts[:, dc * CHUNKS_PER_BATCH + cib : dc * CHUNKS_PER_BATCH + cib + 1],
                    )
                if cib == CHUNKS_PER_BATCH - 1:
                    # ---- per-batch finalize (all fp32) ----
                    # s_sb already holds sum_t(e) per chunk on every partition
                    ssum128 = small.tile([PART, 1], f32, tag="ssum1")
                    nc.vector.reduce_sum(ssum128, s_sb, axis=AX.X)
                    rec = small.tile([PART, 1], f32, tag="rec")
                    nc.vector.reciprocal(rec, ssum128)
                    # ctx4[d_p, dc] = sum_cib ctx_parts ; scale by 1/sum(e)
                    ctx4 = small.tile([PART, DC], f32, tag="ctx4")
                    nc.vector.reduce_sum(
                        ctx4,
                        ctx_parts.rearrange("p (dc cib) -> p dc cib", cib=CHUNKS_PER_BATCH),
                        axis=AX.X,
                    )
                    ctx_sc = small.tile([PART, DC], f32, tag="ctxs")
                    nc.vector.tensor_scalar_mul(ctx_sc, ctx4, rec)
                    # transpose [128, dc] -> [dc, 128] so the output DMA is
                    # 4 contiguous 512B runs
                    ps_o = ps_t.tile([PART, TCHUNK], f32, tag="T", name="ps_o")
                    nc.tensor.transpose(ps_o[0:DC, 0:PART], ctx_sc, ident_f32)
                    ctx_out = outp.tile([DC, PART], f32, tag="ctx_out")
                    nc.vector.tensor_copy(ctx_out, ps_o[0:DC, 0:PART])
                    nc.sync.dma_start(
                        out=out.ap()[b : b + 1, :].rearrange("one (dc p) -> (one dc) p", p=PART),
                        in_=ctx_out,
                    )

            f_state = {}
            ft_state = {}

            def emit_fdma(c):
                # stage 0: HBM -> SBUF with inline fp32->bf16 cast
                sb_ = c // CHUNKS_PER_BATCH
                st0 = (c % CHUNKS_PER_BATCH) * TCHUNK
                if c in preloaded:
                    f_state[c] = preloaded[c]
                    return
                f_chunk = fpool.tile([PART, TILES_PER_CHUNK, D], mdt, tag="F", name="f_chunk")
                nc.gpsimd.dma_start(
                    out=f_chunk,
                    in_=feat.ap()[sb_, st0 : st0 + TCHUNK, :].rearrange(
                        "(j p) d -> p j d", p=PART
                    ),
                )
                f_state[c] = f_chunk

            def emit_ftr_alloc(c):
                ftile_big = ftb.tile([PART, DC, TCHUNK], mdt, tag="FT", name="ftile_big")
                ft_state[c] = (f_state.pop(c), ftile_big)
                return ft_state[c]

            def emit_ftr_piece(st, j):
                # one j-tile: 4 PE transposes (bf16 1 c/col; the fp32
                # preloads transpose at 2 c/col into fp32 PSUM and the DVE
                # repack casts to bf16) + DVE repack
                f_chunk, ftile_big = st
                is_f32 = f_chunk.dtype == f32
                tr_ident = ident_f32 if is_f32 else ident
                ps_tr = ps_t.tile(
                    [PART, TCHUNK], f32 if is_f32 else mdt, tag="T", name="ps_tr"
                )
                for dc in range(DC):
                    nc.tensor.transpose(
                        ps_tr[:, ts(dc, PART)], f_chunk[:, j, ts(dc, PART)], tr_ident
                    )
                nc.vector.tensor_copy(
                    ftile_big[:, :, ts(j, PART)],
                    ps_tr.rearrange("p (c t) -> p c t", c=DC),
                )

            # head: fill the PE with the hid transposes, then chunk 0's
            # transposes, then the rest of setup while W1/W2 land
            for c in range(NPRE):
                emit_fdma(c)
            emit_setup()
            st0 = emit_ftr_alloc(0)
            for j in range(TILES_PER_CHUNK):
                emit_ftr_piece(st0, j)
            emit_setup_b()

            for chunk in range(NCHUNKS + 1):
                # V-dot + exp of the previous chunk lead this chunk
                if prev is not None:
                    emit_scores(prev)

                if chunk + 4 < NCHUNKS:
                    emit_fdma(chunk + 4)
                tr_next = emit_ftr_alloc(chunk + 1) if chunk + 1 < NCHUNKS else None

                if chunk < NCHUNKS:
                    b = chunk // CHUNKS_PER_BATCH
                    cib = chunk % CHUNKS_PER_BATCH
                    _, ftile_big = ft_state.pop(chunk)

                # context stage of the PREVIOUS chunk overlaps this chunk's mains
                if prev is not None:
                    emit_context(prev)
                    prev = None

                if chunk < NCHUNKS:
                    # S2: main matmul + tanh (transposed layout [u, t]);
                    # next chunk's transposes interleave between uc groups so
                    # the DVE repack of piece j overlaps the next mains group
                    tanh_sb = tanhp.tile([PART, UC, TCHUNK], mdt, tag="tanh")
                    for uc in range(UC):
                        ps_f = ps_mm.tile([PART, TCHUNK], f32, tag="mm")
                        for dc in range(DC):
                            nc.tensor.matmul(
                                ps_f,
                                w1_sb[:, dc, ts(uc, PART)],
                                ftile_big[:, dc, :],
                                start=(dc == 0),
                                stop=(dc == DC - 1),
                            )
                        nc.scalar.activation(
                            tanh_sb[:, uc, :],
                            ps_f,
                            ACT.Tanh,
                            bias=bias_cols[:, uc, b : b + 1],
                        )
                        if tr_next is not None:
                            emit_ftr_piece(tr_next, uc)
                    prev = {"b": b, "cib": cib, "tanh": tanh_sb, "ftile_big": ftile_big}
                elif tr_next is not None:
                    for j in range(TILES_PER_CHUNK):
                        emit_ftr_piece(tr_next, j)

    nc.compile()
    _BUILD_CACHE[mm_dt_name] = nc
    return nc


def kernel(**inputs):
    from concourse.bass_utils import run_bass_kernel_spmd

    nc = build_bass()

    feat = np.ascontiguousarray(np.asarray(inputs["features"], dtype=np.float32))
    hid = np.ascontiguousarray(np.asarray(inputs["hidden"], dtype=np.float32))
    shared = {
        k: np.ascontiguousarray(np.asarray(inputs[k], dtype=np.float32))
        for k in ("W1_w", "W1_b", "W2_w", "W2_b", "V_w", "V_b")
    }
    in_maps = []
    for c in range(NCORES):
        m = dict(shared)
        m["features"] = feat[c * BPC : (c + 1) * BPC]
        m["hidden"] = hid[c * BPC : (c + 1) * BPC]
        in_maps.append(m)

    res = run_bass_kernel_spmd(nc, in_maps, list(range(NCORES)))
    return np.concatenate([res.results[c]["context"] for c in range(NCORES)], axis=0)



# revision 7
# speedup vs baseline: 1.0615x; 1.0615x over previous
"""Bahdanau attention kernel for Trainium2 (8 NeuronCores, SPMD data-parallel).

Reference computation (per batch b):
    f_proj = features[b] @ W1_w + W1_b            # [T, U]
    h_proj = hidden[b] @ W2_w + W2_b              # [U]
    score  = tanh(f_proj + h_proj) @ V_w + V_b    # [T]
    attn   = softmax(score)                       # [T]
    context[b] = sum_t attn[t] * features[b, t]   # [D]

Sharding: data-parallel over batch (64 batches / 8 cores = 8 per core),
weights replicated.

Per-core dataflow (bf16 matmul operands, fp32 accumulation everywhere):
  - chunks 0..NPRE-1 of F load as fp32 over the HWDGE queues (per-tile
    256KB DMAs with clean 2KB/partition descriptors), then DVE casts to
    bf16; chunks NPRE.. stream through the gpsimd casting DMA (SWDGE)
    whose pipeline has ~15us startup latency.
  - PE transposes (bf16, LDW-transpose-mode + ident stream) produce
    F^T [128(d), dc, t] in PSUM; DVE repacks to SBUF.
  - main matmul computes f_proj TRANSPOSED: [u(part), t(free)] =
    W1_chunk^T @ F^T, so the (W1_b + h_proj) bias is a per-partition
    scalar that fuses into the ACT Tanh instruction (bf16 out).
  - score uses a REPLICATED stationary V_rep[u, m] = V[u], so the PE
    produces score broadcast across all 128 partitions in one shot.
    ACT Exp turns that into e_bc [128, t] bf16 in SBUF with the per-
    chunk sum(e) accumulated for free.
  - context via DVE fused multiply+reduce over the resident F^T tiles.
  - head: a dummy-matmul warmup stream keeps the PE HAM activity
    monitor busy from ~7us (end of NEFF preamble) so everything runs at
    2.4GHz; small constants arrive host-packed ([128, x] layouts) to
    avoid 4-byte-descriptor DMA storms; W1/W2 load as 4 per-dc
    contiguous slabs interleaved across the two HWDGE rings right
    behind chunk 0.
  - per-batch finalize is emitted AFTER the next chunk's mains so the
    PE never stalls on the DVE finalize chain; the last chunk's
    score/exp/context run in two t-halves to shorten the serial tail.
"""

import sys

for _p in ("/opt/trn_rl_repo", "/opt/pypackages"):
    if _p not in sys.path:
        sys.path.insert(0, _p)

import numpy as np

B, T, D, U = 64, 2048, 512, 512
NCORES = 8
BPC = B // NCORES          # batches per core
PART = 128
DC = D // PART             # 4 contraction chunks
UC = U // PART             # 4 u chunks
TCHUNK = 512               # t columns processed per main-matmul group
TILES_PER_CHUNK = TCHUNK // PART          # 4
NCHUNKS = (BPC * T) // TCHUNK             # 32
CHUNKS_PER_BATCH = T // TCHUNK            # 4
NPRE = 4                   # chunks preloaded as fp32 over HWDGE
WARMUP_MMS = 14            # dummy matmuls to warm the PE HAM clock gate
NSMALL = 13                # host-packed small consts: b1[4] b2[4] v[4] vb[1]

MM_DT_NAME = "bfloat16"    # dtype tag for matmul operands


_BUILD_CACHE = {}


def build_bass(mm_dt_name=MM_DT_NAME):
    """Build + compile the per-core Bass program (same on all cores)."""
    if mm_dt_name in _BUILD_CACHE:
        return _BUILD_CACHE[mm_dt_name]

    import concourse.mybir as mybir
    import concourse.tile as tile
    from concourse import bacc
    from concourse.bass import ts
    from concourse.masks import make_identity

    f32 = mybir.dt.float32
    mdt = getattr(mybir.dt, mm_dt_name)
    ACT = mybir.ActivationFunctionType
    AX = mybir.AxisListType
    ALU = mybir.AluOpType

    nc = bacc.Bacc("TRN2", target_bir_lowering=False, debug=False)

    feat = nc.dram_tensor("features", [BPC, T, D], f32, kind="ExternalInput")
    w1 = nc.dram_tensor("W1_w", [D, U], f32, kind="ExternalInput")
    w2 = nc.dram_tensor("W2_w", [D, U], f32, kind="ExternalInput")
    hidT = nc.dram_tensor("hidT", [PART, DC, BPC], f32, kind="ExternalInput")
    smallp = nc.dram_tensor("smallp", [PART, NSMALL], f32, kind="ExternalInput")
    out = nc.dram_tensor("context", [BPC, D], f32, kind="ExternalOutput")

    with tile.TileContext(nc) as tc:
        with (
            tc.tile_pool(name="consts", bufs=1) as consts,
            tc.tile_pool(name="warm", bufs=1) as warmp,
            tc.tile_pool(name="fpool", bufs=5) as fpool,
            tc.tile_pool(name="fcpool", bufs=3) as fcpool,
            tc.tile_pool(name="fpre32", bufs=NPRE) as fpre32,
            tc.tile_pool(name="ftb", bufs=4) as ftb,
            tc.tile_pool(name="tanh", bufs=3) as tanhp,
            tc.tile_pool(name="small", bufs=3) as small,
            tc.tile_pool(name="ebc", bufs=2) as ebcp,
            tc.tile_pool(name="pscratch", bufs=2) as pscratch,
            tc.tile_pool(name="ctxp", bufs=2) as ctxp,
            tc.tile_pool(name="outp", bufs=2) as outp,
            tc.tile_pool(name="ps_mm", bufs=3, space="PSUM") as ps_mm,
            tc.tile_pool(name="ps_t", bufs=3, space="PSUM") as ps_t,
            tc.tile_pool(name="ps_s", bufs=1, space="PSUM") as ps_s,
            tc.tile_pool(name="ps_w", bufs=1, space="PSUM") as ps_w,
        ):
            # ---------------- PE warmup stream ----------------
            # the HAM clock gate needs ~3.4us of sustained PE activity to
            # lift the PE from 1.2 to 2.4GHz; run dummy matmuls while the
            # head DMAs land so real work starts warm.
            wstat = warmp.tile([PART, PART], mdt)
            nc.vector.memset(wstat, 0.003)
            wmov = warmp.tile([PART, TCHUNK], mdt)
            nc.vector.memset(wmov, 0.007)
            ps_wt = ps_w.tile([PART, TCHUNK], f32, tag="W")
            for k in range(WARMUP_MMS):
                nc.tensor.matmul(
                    ps_wt, wstat, wmov, start=(k == 0), stop=(k == WARMUP_MMS - 1)
                )

            # ---------------- constants / setup ----------------
            ident_f32 = consts.tile([PART, PART], f32)
            make_identity(nc, ident_f32)
            ident = consts.tile([PART, PART], mdt)
            nc.vector.tensor_copy(ident, ident_f32)
            ones128 = consts.tile([PART, PART], f32)
            nc.vector.memset(ones128, 1.0)

            # host-packed small consts: one clean DMA each
            sp_sb = consts.tile([PART, NSMALL], f32)
            nc.sync.dma_start(out=sp_sb, in_=smallp.ap())
            hidT_sb = consts.tile([PART, DC, BPC], f32)
            nc.sync.dma_start(out=hidT_sb, in_=hidT.ap())
            vb_bc = sp_sb[:, 12:13]

            w1_f32 = consts.tile([PART, DC, U], f32)
            w1_sb = consts.tile([PART, DC, U], mdt)
            w2_sb = consts.tile([PART, DC, U], f32)

            f_state = {}

            def emit_fdma(c):
                # stage 0 for SWDGE chunks: HBM -> SBUF with inline cast
                sb_ = c // CHUNKS_PER_BATCH
                st0 = (c % CHUNKS_PER_BATCH) * TCHUNK
                f_chunk = fpool.tile(
                    [PART, TILES_PER_CHUNK, D], mdt, tag="F", name="f_chunk"
                )
                nc.gpsimd.dma_start(
                    out=f_chunk,
                    in_=feat.ap()[sb_, st0 : st0 + TCHUNK, :].rearrange(
                        "(j p) d -> p j d", p=PART
                    ),
                )
                f_state[c] = f_chunk

            # kick the SWDGE pipeline immediately (15-18us startup); further
            # chunks are prefetched strictly in chunk order from the loop so
            # fpool slot reuse never crosses a future chunk's pipeline
            emit_fdma(NPRE)
            emit_fdma(NPRE + 1)

            # fp32 preloads + W1/W2 per-dc slabs, interleaved across the two
            # HWDGE rings; order = critical-path order: c0, W1, W2, c1, c2, c3
            preloaded = {}

            def preload_tiles(pch):
                pb = pch // CHUNKS_PER_BATCH
                pt0 = (pch % CHUNKS_PER_BATCH) * TCHUNK
                f_pre = fpre32.tile(
                    [PART, TILES_PER_CHUNK, D], f32, tag="F32", name=f"f_pre_{pch}"
                )
                tiles = []
                for j in range(TILES_PER_CHUNK):
                    tiles.append(
                        (f_pre[:, j, :],
                         feat.ap()[pb, pt0 + j * PART : pt0 + (j + 1) * PART, :])
                    )
                preloaded[pch] = f_pre
                return tiles

            head_dmas = []
            head_dmas += preload_tiles(0)
            head_dmas += [
                (w1_f32[:, dc, :], w1.ap()[dc * PART : (dc + 1) * PART, :])
                for dc in range(DC)
            ]
            head_dmas += [
                (w2_sb[:, dc, :], w2.ap()[dc * PART : (dc + 1) * PART, :])
                for dc in range(DC)
            ]
            for pch in range(1, NPRE):
                head_dmas += preload_tiles(pch)
            for i, (o, i_) in enumerate(head_dmas):
                eng = nc.sync if i % 2 == 0 else nc.scalar
                eng.dma_start(out=o, in_=i_)

            # DVE-order-sensitive: chunk-0 casts first (critical path), then
            # W1 casts, then the small-const math.
            def emit_fcast(c):
                # preloaded fp32 chunk -> bf16 f_chunk (DVE, 4x per-j copies)
                f_pre = preloaded.pop(c)
                f_chunk = fcpool.tile(
                    [PART, TILES_PER_CHUNK, D], mdt, tag="FC", name="f_cast"
                )
                for j in range(TILES_PER_CHUNK):
                    nc.vector.tensor_copy(f_chunk[:, j, :], f_pre[:, j, :])
                f_state[c] = f_chunk

            emit_fcast(0)
            for dc in range(DC):
                nc.vector.tensor_copy(w1_sb[:, dc, :], w1_f32[:, dc, :])

            b12_sb = consts.tile([PART, UC], f32)
            nc.vector.tensor_add(b12_sb, sp_sb[:, 0:UC], sp_sb[:, UC : 2 * UC])
            # V replicated across the stationary free dim: the score matmul
            # then emits score broadcast over all 128 output partitions
            v_rep = consts.tile([PART, UC, PART], mdt)
            for uc in range(UC):
                nc.vector.tensor_scalar_mul(
                    v_rep[:, uc, :], ones128, sp_sb[:, 2 * UC + uc : 2 * UC + uc + 1]
                )
            bias_cols = consts.tile([PART, UC, BPC], f32)

            def emit_setup_b():
                # h_projT[u, b] = sum_dc W2[dc]^T @ hiddenT[dc]  (+W2_b+W1_b)
                for uc in range(UC):
                    ps_h = ps_t.tile([PART, TCHUNK], f32, tag="T", name="ps_h2")
                    for dc in range(DC):
                        nc.tensor.matmul(
                            ps_h[:, 0:BPC],
                            w2_sb[:, dc, ts(uc, PART)],
                            hidT_sb[:, dc, :],
                            start=(dc == 0),
                            stop=(dc == DC - 1),
                        )
                    nc.vector.tensor_scalar_add(
                        bias_cols[:, uc, :], ps_h[:, 0:BPC], b12_sb[:, uc : uc + 1]
                    )

            # ---------------- main loop ----------------
            prev = None          # chunk state awaiting its score/context stage
            batch_state = {}     # per-batch running-sum / ctx accumulators
            SC = CHUNKS_PER_BATCH + 1   # extra column for the split tail

            def alloc_batch_state():
                s_sb = small.tile([PART, SC], f32, tag="ssum", name="s_sb")
                ctx_parts = ctxp.tile([PART, DC, SC], f32, tag="ctxp", name="ctx_parts")
                nc.vector.memset(s_sb[:, SC - 1 : SC], 0.0)
                nc.vector.memset(ctx_parts[:, :, SC - 1 : SC], 0.0)
                batch_state["s_sb"] = s_sb
                batch_state["ctx_parts"] = ctx_parts

            def emit_scores(st, split=False):
                b, cib = st["b"], st["cib"]
                if cib == 0:
                    alloc_batch_state()
                s_sb = batch_state["s_sb"]
                # score broadcast [128, t]: every output partition m gets
                # score[t] because the stationary V_rep column m is V itself
                ps_sc = ps_s.tile([PART, TCHUNK], f32, tag="score")
                e_bc = ebcp.tile([PART, TCHUNK], mdt, tag="e_bc")
                halves = 2 if split else 1
                hw = TCHUNK // halves
                for h in range(halves):
                    sl = slice(h * hw, (h + 1) * hw)
                    for uc in range(UC):
                        nc.tensor.matmul(
                            ps_sc[:, sl],
                            v_rep[:, uc, :],
                            st["tanh"][:, uc, sl],
                            start=(uc == 0),
                            stop=(uc == UC - 1),
                        )
                    # e = exp(score + V_b) on all 128 partitions -> SBUF bf16;
                    # the ACT accumulator gives sum_t(e) per partition for free
                    nc.scalar.activation(
                        e_bc[:, sl],
                        ps_sc[:, sl],
                        ACT.Exp,
                        bias=vb_bc,
                        accum_out=s_sb[:, cib + h : cib + h + 1],
                    )
                st["e_bc"] = e_bc

            def emit_context_stt(st, split=False):
                b, cib = st["b"], st["cib"]
                ctx_parts = batch_state["ctx_parts"]
                e_bc = st["e_bc"]
                ftile_big = st["ftile_big"]
                halves = 2 if split else 1
                hw = TCHUNK // halves
                # DVE fused multiply + free-dim reduce (all-bf16 -> 2x mode):
                # ctx_parts[d, dc, cib] = sum_t F^T[d, dc, t] * e[t]
                for h in range(halves):
                    sl = slice(h * hw, (h + 1) * hw)
                    for dc in range(DC):
                        scr = pscratch.tile([PART, TCHUNK], mdt, tag="scr", name="pscr")
                        nc.vector.scalar_tensor_tensor(
                            out=scr[:, sl],
                            in0=ftile_big[:, dc, sl],
                            scalar=1.0,
                            in1=e_bc[:, sl],
                            op0=ALU.mult,
                            op1=ALU.mult,
                            accum_out=ctx_parts[:, dc, cib + h : cib + h + 1],
                        )

            def emit_finalize(fin):
                b, s_sb, ctx_parts = fin["b"], fin["s_sb"], fin["ctx_parts"]
                # s_sb already holds sum_t(e) per chunk on every partition
                ssum128 = small.tile([PART, 1], f32, tag="ssum1")
                nc.vector.reduce_sum(ssum128, s_sb, axis=AX.X)
                rec = small.tile([PART, 1], f32, tag="rec")
                nc.vector.reciprocal(rec, ssum128)
                # ctx4[d_p, dc] = sum_cib ctx_parts ; scale by 1/sum(e)
                ctx4 = small.tile([PART, DC], f32, tag="ctx4")
                nc.vector.reduce_sum(ctx4, ctx_parts, axis=AX.X)
                ctx_sc = small.tile([PART, DC], f32, tag="ctxs")
                nc.vector.tensor_scalar_mul(ctx_sc, ctx4, rec)
                # transpose [128, dc] -> [dc, 128] so the output DMA is
                # 4 contiguous 512B runs
                ps_o = ps_t.tile([PART, TCHUNK], f32, tag="T", name="ps_o")
                nc.tensor.transpose(ps_o[0:DC, 0:PART], ctx_sc, ident_f32)
                ctx_out = outp.tile([DC, PART], f32, tag="ctx_out")
                nc.vector.tensor_copy(ctx_out, ps_o[0:DC, 0:PART])
                nc.sync.dma_start(
                    out=out.ap()[b : b + 1, :].rearrange(
                        "one (dc p) -> (one dc) p", p=PART
                    ),
                    in_=ctx_out,
                )

            ft_state = {}

            def emit_ftr_alloc(c):
                if c in preloaded:
                    emit_fcast(c)
                ftile_big = ftb.tile([PART, DC, TCHUNK], mdt, tag="FT", name="ftile_big")
                ft_state[c] = (f_state.pop(c), ftile_big)
                return ft_state[c]

            def emit_ftr_piece(st, j):
                # one j-tile: 4 PE transposes (bf16; LDW streams the data in
                # transpose mode, MM streams ident) + DVE repack to SBUF
                f_chunk, ftile_big = st
                ps_tr = ps_t.tile([PART, TCHUNK], mdt, tag="T", name="ps_tr")
                for dc in range(DC):
                    nc.tensor.transpose(
                        ps_tr[:, ts(dc, PART)], f_chunk[:, j, ts(dc, PART)], ident
                    )
                nc.vector.tensor_copy(
                    ftile_big[:, :, ts(j, PART)],
                    ps_tr.rearrange("p (c t) -> p c t", c=DC),
                )

            # head: chunk 0 transposes ready the first mains; setup_b sits
            # between them (W2 arrives right behind W1)
            st0 = emit_ftr_alloc(0)
            for j in range(TILES_PER_CHUNK):
                emit_ftr_piece(st0, j)
            emit_setup_b()

            pending_finalize = None
            for chunk in range(NCHUNKS + 1):
                last = chunk == NCHUNKS
                # V-dot + exp of the previous chunk lead this chunk
                if prev is not None:
                    emit_scores(prev, split=last)

                if chunk + NPRE + 2 < NCHUNKS:
                    emit_fdma(chunk + NPRE + 2)
                tr_next = emit_ftr_alloc(chunk + 1) if chunk + 1 < NCHUNKS else None

                if not last:
                    b = chunk // CHUNKS_PER_BATCH
                    cib = chunk % CHUNKS_PER_BATCH
                    _, ftile_big = ft_state.pop(chunk)

                # context stage of the PREVIOUS chunk overlaps this chunk's mains
                if prev is not None:
                    emit_context_stt(prev, split=last)
                    if prev["cib"] == CHUNKS_PER_BATCH - 1:
                        pending_finalize = {
                            "b": prev["b"],
                            "s_sb": batch_state["s_sb"],
                            "ctx_parts": batch_state["ctx_parts"],
                        }
                    prev = None

                if not last:
                    # S2: main matmul + tanh (transposed layout [u, t]);
                    # next chunk's transposes interleave between uc groups so
                    # the DVE repack of piece j overlaps the next mains group
                    tanh_sb = tanhp.tile([PART, UC, TCHUNK], mdt, tag="tanh")
                    for uc in range(UC):
                        ps_f = ps_mm.tile([PART, TCHUNK], f32, tag="mm")
                        for dc in range(DC):
                            nc.tensor.matmul(
                                ps_f,
                                w1_sb[:, dc, ts(uc, PART)],
                                ftile_big[:, dc, :],
                                start=(dc == 0),
                                stop=(dc == DC - 1),
                            )
                        nc.scalar.activation(
                            tanh_sb[:, uc, :],
                            ps_f,
                            ACT.Tanh,
                            bias=bias_cols[:, uc, b : b + 1],
                        )
                        if tr_next is not None:
                            emit_ftr_piece(tr_next, uc)
                    prev = {"b": b, "cib": cib, "tanh": tanh_sb, "ftile_big": ftile_big}
                elif tr_next is not None:
                    for j in range(TILES_PER_CHUNK):
                        emit_ftr_piece(tr_next, j)

                # deferred: per-batch finalize AFTER this chunk's mains so the
                # PE doesn't stall on the DVE finalize chain
                if pending_finalize is not None:
                    emit_finalize(pending_finalize)
                    pending_finalize = None

    nc.compile()
    _BUILD_CACHE[mm_dt_name] = nc
    return nc


def make_core_inputs(inputs, c):
    """Host-side shard + layout prep for core c (pure numpy, layout only)."""
    f32 = np.float32
    feat = np.ascontiguousarray(np.asarray(inputs["features"][c * BPC : (c + 1) * BPC], dtype=f32))
    hid = np.asarray(inputs["hidden"][c * BPC : (c + 1) * BPC], dtype=f32)
    # hidT[p, dc, b] = hidden[b, dc*128 + p]
    hidT = np.ascontiguousarray(hid.reshape(BPC, DC, PART).transpose(2, 1, 0))
    sp = np.zeros((PART, NSMALL), dtype=f32)
    sp[:, 0:UC] = np.asarray(inputs["W1_b"], dtype=f32).reshape(UC, PART).T
    sp[:, UC : 2 * UC] = np.asarray(inputs["W2_b"], dtype=f32).reshape(UC, PART).T
    sp[:, 2 * UC : 3 * UC] = np.asarray(inputs["V_w"], dtype=f32).reshape(UC, PART).T
    sp[:, 3 * UC] = np.asarray(inputs["V_b"], dtype=f32)[0]
    return {
        "features": feat,
        "W1_w": np.ascontiguousarray(np.asarray(inputs["W1_w"], dtype=f32)),
        "W2_w": np.ascontiguousarray(np.asarray(inputs["W2_w"], dtype=f32)),
        "hidT": hidT,
        "smallp": sp,
    }


def kernel(**inputs):
    from concourse.bass_utils import run_bass_kernel_spmd

    nc = build_bass()
    in_maps = [make_core_inputs(inputs, c) for c in range(NCORES)]
    res = run_bass_kernel_spmd(nc, in_maps, list(range(NCORES)))
    return np.concatenate([res.results[c]["context"] for c in range(NCORES)], axis=0)
